# revision 55
# baseline (speedup 1.0000x reference)
"""CTC loss (keras ctc_batch_cost semantics) as a Bass/Tile kernel on 8
TRN2 NeuronCores.

Strategy (per core, 64 examples):
  - Linear-space CTC forward DP reformulated as a wavefront over the 65
    extended states; each state's full time series is ONE DVE
    tensor_tensor_scan (state = (inflow[t-1] + state) * p[t]).
  - Time is split fwd/bwd: partition rows 0..63 run the forward DP over
    t in [0,256) and rows 64..127 run the backward DP over t in [256,512)
    (s- and t-reversed so every instruction is uniform across partitions).
    Host combines the two halves per example.
  - Gather: per example ONE DMA loads yp[e] as [128, 4*128]; PE transposes
    the four 128x128 blocks to [c, t] in PSUM.  The backward half is
    time-reversed ON THE PE by transposing against an anti-diagonal
    identity (out[c, j] = slab[127-j, c]) with the two t-blocks swapped,
    so no reversed (element-strided) DMA ever happens.
  - Per-(example,dir) probability series: PE matmul (fp32r, 1 cyc/row)
    of a K-scaled one-hot [c, 33] (32 odd-state labels + blank) against
    slabT -> PSUM [33, 256]; one contiguous DMA flattens it into that
    example's partition row of the compact p-store (33 blocks: odd states
    + one shared blank block used by all even states).
  - Scaling: constant K = e^4.55 per step keeps the fp32 DP in range for
    256 steps; host removes T*log(K) at the end.
"""
import contextlib
import ctypes
import sys
import types

import numpy as np

sys.path.insert(0, "/opt/trn_rl_repo")

B, T, C, L = 512, 512, 128, 32
BLANK = C - 1
S = 2 * L + 1            # 65 extended states
TH = T // 2              # 256 timesteps per direction
NCORES = 8
EX_PER_CORE = B // NCORES  # 64
K_VAL = 94.5             # per-step scale (~e^4.55); exact in bf16/tf32 grids
KLOG = float(np.log(np.float64(K_VAL)))
BLK = TH + 1             # alpha-store block stride (guard col + 256)
NS = L + 1               # compact p-store blocks: 32 odd labels + blank
GW = 97                  # gather-matmul stationary width: fwd states at
                         # partitions [0,33), zeros, bwd states at [64,97)
                         # (compute-engine APs must start on a quadrant)


# ---------------------------------------------------------------------------
# axon runtime shims (NTFF profile hook + no-op artifact upload)
# ---------------------------------------------------------------------------
_SO_PATH = "/opt/axon/libaxon_pjrt.so"


def _make_ntff_hook():
    try:
        lib = ctypes.CDLL(_SO_PATH)
    except OSError:
        return None
    if not hasattr(lib, "axon_start_nrt_profile"):
        return None
    lib.axon_start_nrt_profile.argtypes = [
        ctypes.POINTER(ctypes.c_int64),
        ctypes.c_size_t,
    ]
    lib.axon_start_nrt_profile.restype = ctypes.c_int64
    lib.axon_stop_nrt_profile.argtypes = [ctypes.c_char_p]
    lib.axon_stop_nrt_profile.restype = ctypes.c_int64

    @contextlib.contextmanager
    def _hook(output_dir, device_ids):
        import jax

        jax.devices()
        if device_ids:
            ids = (ctypes.c_int64 * len(device_ids))(*device_ids)
            rc = lib.axon_start_nrt_profile(ids, len(device_ids))
        else:
            rc = lib.axon_start_nrt_profile(None, 0)
        if rc != 0:
            raise RuntimeError(f"axon_start_nrt_profile rc={rc}")
        try:
            yield
        finally:
            lib.axon_stop_nrt_profile(str(output_dir).encode())

    return _hook


def _install_shims():
    if "antenv.axon_hooks" not in sys.modules:
        mod = types.ModuleType("antenv.axon_hooks")
        hook = _make_ntff_hook()
        mod.get_axon_ntff_profile_hook = lambda: hook
        mod.set_axon_ntff_profile_hook = lambda h: None
        sys.modules["antenv.axon_hooks"] = mod
    import concourse.bass_utils as bu

    bu.upload_artifacts = lambda tmpdir: str(tmpdir)


# ---------------------------------------------------------------------------
# device program
# ---------------------------------------------------------------------------
_NC_CACHE = {}


def build_program():
    _install_shims()
    import concourse.bacc as bacc
    import concourse.mybir as mybir
    from concourse.masks import make_identity
    from concourse.tile import TileContext

    F32 = mybir.dt.float32
    BF16 = mybir.dt.bfloat16
    ALU = mybir.AluOpType

    nc = bacc.Bacc("TRN2")
    # yp is pre-arranged on the host: yp[tp, e, t4, c] = y_pred[e, 128*t4+tp, c]
    # in bf16, so the full per-core slab loads with two contiguous DMAs.
    yp = nc.dram_tensor(
        "yp", [128, EX_PER_CORE * 4 * C], BF16, kind="ExternalInput"
    )
    oh = nc.dram_tensor(
        "oh", [C, EX_PER_CORE * GW], BF16, kind="ExternalInput"
    )
    msk = nc.dram_tensor("msk", [128, S], BF16, kind="ExternalInput")
    w_out = nc.dram_tensor("W", [128, S], F32, kind="ExternalOutput")

    with TileContext(nc) as tc:
        with (
            tc.tile_pool(name="persist", bufs=1) as persist,
            tc.tile_pool(name="stage", bufs=6) as stage,
            tc.tile_pool(name="upool", bufs=2) as upool,
            tc.tile_pool(name="pp", bufs=4, space="PSUM") as pp,
        ):
            pstore = persist.tile([128, NS * TH], BF16, tag="pstore")
            astore = persist.tile([128, (S + 2) * BLK], BF16, tag="astore")
            msk_sb = persist.tile([128, S], BF16, tag="msk")
            ident = persist.tile([128, 128], BF16, tag="ident")
            antid = persist.tile([128, 128], BF16, tag="antid")
            wout_sb = persist.tile([128, S], F32, tag="wout")
            oh_all = persist.tile([C, EX_PER_CORE * GW], BF16, tag="oh_all")
            # yp staged as independent tiles, front-loaded small so the very
            # first transposes wait only on a 4-example DMA.
            CH = [4, 12, 24, 24]
            CUM = [0]
            for c in CH:
                CUM.append(CUM[-1] + c)
            yp_q = [
                persist.tile(
                    [128, CH[q] * 4 * C], BF16, name=f"yp_q{q}", tag=f"yp_q{q}"
                )
                for q in range(len(CH))
            ]
            for q in range(len(CH)):
                nc.sync.dma_start(
                    yp_q[q][:, :],
                    yp[:, CUM[q] * 4 * C : CUM[q + 1] * 4 * C],
                )
            nc.sync.dma_start(msk_sb[:, :], msk[:, :])
            nc.scalar.dma_start(oh_all[:, :], oh[:, :])
            make_identity(nc, ident[:, :])
            # anti-diagonal identity: antid[x, y] = 1 iff x + y == 127
            nc.gpsimd.memset(antid[:, :], 0.0)
            nc.gpsimd.affine_select(
                out=antid[:, :],
                in_=antid[:, :],
                compare_op=ALU.not_equal,
                fill=1.0,
                base=-127,
                pattern=[[1, 128]],
                channel_multiplier=1,
            )

            # alpha store init: zero the two inflow-guard blocks and the
            # guard column of every output block; backward rows get guard
            # value 1.0 on output blocks 0 and 1 (end states 64, 63).
            ablocks = astore[:, :].rearrange("p (s c) -> p s c", c=BLK)
            nc.vector.memset(astore[:, : 2 * BLK], 0.0)
            nc.vector.memset(ablocks[:, 2 : S + 2, 0:1], 0.0)
            nc.vector.memset(astore[64:128, 2 * BLK : 2 * BLK + 1], 1.0)
            nc.vector.memset(astore[64:128, 3 * BLK : 3 * BLK + 1], 1.0)

            # ---------------- gather phase ----------------
            # Software-pipelined: transposes for example r+1 are emitted
            # BEFORE the matmul of example r, so the PE's in-order queue
            # never puts transposes behind the matmul they must precede,
            # and the Act slabT copy of r+1 overlaps the matmul of r.
            _PEND = [None]
            _TPS = {}

            def emit_transposes(rr):
                qq = next(i for i in range(len(CH)) if CUM[i + 1] > rr)
                ypt = yp_q[qq]
                sb = (rr - CUM[qq]) * 4 * C
                tps = pp.tile(
                    [128, 2 * TH], BF16, name=f"tps{rr}", tag="slabT"
                )
                # fwd half: natural order into cols [0, 256)
                nc.tensor.transpose(
                    tps[:, 0:128], ypt[:, sb : sb + C], ident[:, :]
                )
                nc.tensor.transpose(
                    tps[:, 128:256], ypt[:, sb + C : sb + 2 * C], ident[:, :]
                )
                # bwd half: anti-identity time-reverses each block; block
                # order swapped so cols [256, 512) run t=511..256.
                nc.tensor.transpose(
                    tps[:, 256:384],
                    ypt[:, sb + 3 * C : sb + 4 * C],
                    antid[:, :],
                )
                nc.tensor.transpose(
                    tps[:, 384:512],
                    ypt[:, sb + 2 * C : sb + 3 * C],
                    antid[:, :],
                )
                _TPS[rr] = tps

            emit_transposes(0)
            emit_transposes(1)
            for r in range(EX_PER_CORE):
                    if r + 2 < EX_PER_CORE:
                        emit_transposes(r + 2)
                    slabT_ps = _TPS.pop(r)
                    slabT = stage.tile([128, 2 * TH], BF16, tag="slabT_sb")
                    nc.scalar.copy(slabT[:, :], slabT_ps[:, :])
                    # one fused matmul: [c, 97]^T @ [c, 512] -> [97, 512];
                    # only the [0:33, 0:256] and [64:97, 256:512] blocks
                    # are meaningful.
                    gout_ps = pp.tile([GW, 2 * TH], F32, tag="gout")
                    nc.tensor.matmul(
                        gout_ps[:, :],
                        oh_all[:, r * GW : (r + 1) * GW],
                        slabT[:, :],
                        start=True,
                        stop=True,
                    )
                    # flush the PREVIOUS example's bwd scatter: by now its
                    # data is long ready, so this DMA never stalls the Act
                    # sequencer on a semaphore wait.
                    if _PEND[0] is not None:
                        pr, pgsb = _PEND[0]
                        nc.scalar.dma_start(
                            pstore[
                                EX_PER_CORE + pr : EX_PER_CORE + pr + 1, :
                            ].rearrange("a (s t) -> a s t", t=TH),
                            pgsb[64 : 64 + NS, :],
                        )
                    gsb = stage.tile([128, TH], BF16, tag="gout_sb")
                    nc.vector.tensor_copy(gsb[0:NS, :], gout_ps[0:NS, 0:TH])
                    nc.vector.tensor_copy(
                        gsb[64 : 64 + NS, :],
                        gout_ps[64 : 64 + NS, TH : 2 * TH],
                    )
                    # fwd scatter on the sync queue: its sem-stalls are free
                    # (nothing else is pending there).
                    nc.sync.dma_start(
                        pstore[r : r + 1, :].rearrange(
                            "a (s t) -> a s t", t=TH
                        ),
                        gsb[0:NS, :],
                    )
                    _PEND[0] = (r, gsb)

            # flush the last example's bwd scatter
            pr, pgsb = _PEND[0]
            nc.scalar.dma_start(
                pstore[
                    EX_PER_CORE + pr : EX_PER_CORE + pr + 1, :
                ].rearrange("a (s t) -> a s t", t=TH),
                pgsb[64 : 64 + NS, :],
            )

            # ---------------- wavefront ----------------
            for i in range(S):
                if i % 2 == 1:
                    # odd (label) state: inflow needs the masked skip term
                    u = upool.tile([128, TH], BF16, tag="u")
                    nc.vector.scalar_tensor_tensor(
                        u[:, :],
                        astore[:, i * BLK : i * BLK + TH],
                        msk_sb[:, i : i + 1],
                        astore[:, (i + 1) * BLK : (i + 1) * BLK + TH],
                        ALU.mult,
                        ALU.add,
                    )
                    inflow = u[:, :]
                    pb = (i - 1) // 2
                else:
                    # even (blank) state: skip mask is all-zero; inflow is
                    # just the previous state's series, read in place.
                    inflow = astore[:, (i + 1) * BLK : (i + 1) * BLK + TH]
                    pb = L
                ob = (i + 2) * BLK
                nc.vector.tensor_tensor_scan(
                    astore[:, ob + 1 : ob + 1 + TH],
                    inflow,
                    pstore[:, pb * TH : (pb + 1) * TH],
                    1.0 if i < 2 else 0.0,
                    ALU.add,
                    ALU.mult,
                )

            # boundary column t = TH-1 of every state -> compact tile -> out
            nc.vector.tensor_copy(
                wout_sb[:, :].rearrange("p (s o) -> p s o", o=1),
                ablocks[:, 2 : 2 + S, TH : TH + 1],
            )
            nc.sync.dma_start(w_out[:, :], wout_sb[:, :])

    nc.finalize()
    return nc


def _get_program():
    if "nc" not in _NC_CACHE:
        _NC_CACHE["nc"] = build_program()
    return _NC_CACHE["nc"]


# ---------------------------------------------------------------------------
# host side
# ---------------------------------------------------------------------------
def _host_prep(y_true, y_pred):
    y_true = np.asarray(y_true)
    y_pred = np.ascontiguousarray(np.asarray(y_pred, dtype=np.float32))
    ext = np.full((B, S), BLANK, np.int64)
    ext[:, 1::2] = y_true.astype(np.int64)
    skip = np.zeros((B, S), bool)
    skip[:, 2:] = (ext[:, 2:] != BLANK) & (ext[:, 2:] != ext[:, :-2])
    K = np.float32(K_VAL)

    import ml_dtypes

    BF = ml_dtypes.bfloat16
    in_maps = []
    for k in range(NCORES):
        sl = slice(k * EX_PER_CORE, (k + 1) * EX_PER_CORE)
        lab = y_true[sl].astype(np.int64)          # [64, 32]
        # compact one-hot: cols [0,32) -> fwd labels, col 32 -> blank,
        # cols [64,96) -> reversed labels, col 96 -> blank; zeros between.
        # Stored [C, EX*GW] so the device loads it with one contiguous DMA.
        ohk = np.zeros((EX_PER_CORE, C, GW), np.float32)
        r_idx = np.arange(EX_PER_CORE)[:, None]
        j_idx = np.arange(L)[None, :]
        ohk[r_idx, lab, j_idx] = K
        ohk[:, BLANK, L] = K
        ohk[r_idx, lab[:, ::-1], 64 + j_idx] = K
        ohk[:, BLANK, 64 + L] = K
        ohk = np.ascontiguousarray(
            ohk.transpose(1, 0, 2).reshape(C, EX_PER_CORE * GW)
        ).astype(BF)
        # device layout: yp_dev[tp, ((e*4)+t4)*C + c] = y_pred[e, 128*t4+tp, c]
        ypk = (
            y_pred[sl]
            .reshape(EX_PER_CORE, 4, 128, C)
            .transpose(2, 0, 1, 3)
            .reshape(128, EX_PER_CORE * 4 * C)
        )
        ypk = np.ascontiguousarray(ypk).astype(BF)
        mskk = np.zeros((128, S), np.float32)
        mskk[:EX_PER_CORE] = skip[sl].astype(np.float32)
        # backward rows: iteration i targets state 64-i; its skip inflow
        # comes from state 66-i (mask skip[66-i], zero when out of range).
        sk = np.zeros((EX_PER_CORE, S), np.float32)
        sk[:, : S - 2] = skip[sl, 2:].astype(np.float32)
        mskk[EX_PER_CORE:] = sk[:, ::-1]
        mskk = mskk.astype(BF)
        in_maps.append(
            {
                "yp": ypk,
                "oh": ohk,
                "msk": mskk,
            }
        )
    return in_maps, ext, skip


def _host_combine(Ws, skip):
    loss = np.zeros((B, 1), np.float32)
    for k in range(NCORES):
        Wk = Ws[k].astype(np.float64)
        for r in range(EX_PER_CORE):
            e = k * EX_PER_CORE + r
            wf = Wk[r]                       # alpha[s, 255]
            wb = Wk[EX_PER_CORE + r][::-1]   # B[s, 256]
            a2 = wf.copy()
            a2[1:] += wf[:-1]
            a2[2:] += np.where(skip[e, 2:], wf[:-2], 0.0)
            ptot = float((a2 * wb).sum())
            loss[e, 0] = -(np.log(ptot) - T * KLOG)
    return loss


def kernel(y_true, y_pred, trace=False):
    _install_shims()
    from concourse.bass_utils import run_bass_kernel_spmd

    nc = _get_program()
    in_maps, ext, skip = _host_prep(y_true, y_pred)
    res = run_bass_kernel_spmd(
        nc, in_maps, list(range(NCORES)), trace=trace
    )
    Ws = [res.results[k]["W"] for k in range(NCORES)]
    loss = _host_combine(Ws, skip)
    if trace:
        kernel.last_exec_time_ns = res.exec_time_ns
    return loss
